# revision 20
# baseline (speedup 1.0000x reference)
"""Trainium2 Bass kernel for nn_FFNNTransducerModel (RNN-T style transducer).

Strategy (v2)
-------------
The output grid [B, T, U+1, V] is ragged: only t < enc_size[b], u <= tgt_size[b]
is nonzero (the reference multiplies by that mask).

  host:   - prediction network (embedding + 2-layer MLP + Wp proj + bj1)
            -> per-(b,u) bias vector bvec[b,u,512]        (tiny)
          - encoder projection encp = enc @ We  -> [B, T, 512]  (one GEMM)
          - decompose each example's valid t-tiles into width-3 and width-1
            tile chunks, LPT-pack (chunk, u) items across the 8 cores into
            two fixed grids (SPMD: one program, per-core data)
  device: - per item: h[jc] = relu(encp_T[jc, t-range] + bvec[u])  (DVE/ACT)
          - joint GEMM: psum[v, t*] += wj2[jc].T @ h[jc]  (fp32 PSUM accum)
          - epilogue: copy psum -> SBUF bf16, DMA out  (bj2 added on host)
  host:   - out.astype(f32) + bj2, scatter item tiles (transposed) into the
            zero-initialized output; invalid region stays exactly 0.

All device tensors are bf16 (fp32 PSUM accumulation).  The compiled program
depends only on the grid shape, which is derived from input sizes and cached.
"""

import math
import os
import sys
import types

import numpy as np

import concourse.bass as bass
import concourse.mybir as mybir
import concourse.tile as tile
from concourse import bass_utils

F32 = mybir.dt.float32
BF16 = mybir.dt.bfloat16
P = 128

# Model dims (fixed by the problem)
B, T, U, V = 8, 512, 64, 128
ENC, PRED, JOIN, EMB, H = 512, 256, 512, 128, 2
NU = U + 1  # 65

NWARM = int(os.environ.get("KERNEL_NWARM", "6"))
# measured per-op engine costs (ns) used by the static balancer: v, s, g
C_HA = [float(x) for x in os.environ.get("KERNEL_C_HA", "375,465,1600").split(",")]
C_HB = [float(x) for x in os.environ.get("KERNEL_C_HB", "295,380,900").split(",")]
C_EPA = [float(x) for x in os.environ.get("KERNEL_C_EPA", "1085,1070,1e9").split(",")]
C_EPB = [float(x) for x in os.environ.get("KERNEL_C_EPB", "790,823,1e9").split(",")]
# Pool (gpsimd) h-op probe: at most this many A-chunks go to Pool
POOL_MAX = int(os.environ.get("KERNEL_POOL_MAX", "4"))

_CACHE = {}


def _install_ntff_hook():
    """The image's antenv lacks axon_hooks; shim it so trace=True works."""
    if "antenv.axon_hooks" in sys.modules:
        return
    mod = types.ModuleType("antenv.axon_hooks")
    _hook = [None]
    mod.set_axon_ntff_profile_hook = lambda h: _hook.__setitem__(0, h)
    mod.get_axon_ntff_profile_hook = lambda: _hook[0]
    sys.modules["antenv.axon_hooks"] = mod
    try:
        from trn_agent_boot.trn_boot import _ntff_profile_via_ctypes

        mod.set_axon_ntff_profile_hook(
            _ntff_profile_via_ctypes("/opt/axon/libaxon_pjrt.so")
        )
    except Exception:
        pass


def _split_excess_waits(nc, max_waits=1):
    """This container's walrus supports only one embedded sync-wait per
    instruction; split extras into standalone EventSemaphore waits placed
    immediately before the consumer on the same engine stream."""
    f = nc.m.functions[0]
    for blk in f.blocks:
        insts = list(blk.instructions)
        out = []
        changed = False
        for ins in insts:
            si = getattr(ins, "sync_info", None)
            if si is not None and si.on_wait is not None and len(si.on_wait) > max_waits:
                waits = list(si.on_wait)
                keep, excess = waits[:max_waits], waits[max_waits:]
                for j, w in enumerate(excess):
                    es = mybir.InstEventSemaphore(
                        name=f"{ins.name}_xw{j}",
                        engine=ins.engine,
                        sync_info=mybir.SyncInfo(on_wait=[w], on_update=[]),
                    )
                    out.append(es)
                si.on_wait = keep
                changed = True
            out.append(ins)
        if changed:
            blk.instructions = out
    return nc


def _build_nc(SA, CA, SB, CB):
    """Uniform SPMD program; all data dependence lives in the input arrays.

    Section A: SA slots of width WA=3 t-tiles, CA items (u values) each.
    Section B: SB slots of width 1 t-tile, CB items each, grouped by 4
    into one PSUM bank (CB % 4 == 0).  B groups are interleaved among the
    A items so scalar-engine work doesn't pile up at the end.

    All DRAM inputs are pre-arranged on the host into the exact SBUF
    layout ([128, free]) so every DMA is one descriptor per partition."""
    WA = 3
    WAP = WA * P
    NITA = SA * CA
    NA = max(NITA, 1)
    NGB = (SB * CB) // 4 if SB else 0
    PAIRED = NITA > 0 and NITA % 2 == 0
    NPAIR = NA // 2 if PAIRED else NA

    nc = bass.Bass()
    encpA = nc.dram_tensor("encpA", [SA, P, 4 * WAP], BF16, kind="ExternalInput")
    bvA = nc.dram_tensor("bvA", [P, 4 * NA], F32, kind="ExternalInput")
    if SB:
        SBP = SB * P
        NB = SB * CB
        encpB = nc.dram_tensor("encpB", [P, 4 * SBP], BF16, kind="ExternalInput")
        bvB = nc.dram_tensor("bvB", [P, 4 * NB], F32, kind="ExternalInput")
    wj2 = nc.dram_tensor("wj2", [P, 4 * V], BF16, kind="ExternalInput")
    outA = nc.dram_tensor(
        "outA", [NPAIR, P, (2 if PAIRED else 1) * WAP], BF16, kind="ExternalOutput"
    )
    if SB:
        outB = nc.dram_tensor("outB", [NGB, P, 4 * P], BF16, kind="ExternalOutput")

    with tile.TileContext(nc) as tc:
        with (
            tc.tile_pool(name="consts", bufs=1) as consts,
            tc.tile_pool(name="encpp", bufs=2) as encpp,
            tc.tile_pool(name="encbp", bufs=1) as encbp,
            tc.tile_pool(name="hp", bufs=12) as hp,
            tc.tile_pool(name="outp", bufs=5) as outp,
            tc.tile_pool(name="psj", bufs=4, space="PSUM") as psj,
        ):
            # warm tile init on Pool — its queue is free earliest, and the
            # PE warmup matmuls can then start during the DMA-wait window
            warm = consts.tile([P, 4 * P], BF16, tag="warm")
            nc.gpsimd.memset(warm[:], 0.0)
            warm2 = consts.tile([P, P], BF16, tag="warm2")

            def load_slot(s, split=False, ring=None):
                t = encpp.tile([P, 4 * WAP], BF16, tag="encp")
                dma = (ring or nc.sync).dma_start
                if split:
                    dma(t[:, : 2 * WAP], encpA[s][:, : 2 * WAP])
                    dma(t[:, 2 * WAP :], encpA[s][:, 2 * WAP :])
                else:
                    dma(t[:], encpA[s])
                return t

            # critical first loads on the SP ring (shortest preamble);
            # everything else on the Pool ring; ACT ring stays free for h-ops
            enc_cur = load_slot(0, split=True) if SA else None
            bvA_all = consts.tile([P, 4 * NA], F32, tag="bvA")
            nc.sync.dma_start(bvA_all[:], bvA[:, :])
            bvA_sb = [bvA_all[:, jc * NA : (jc + 1) * NA] for jc in range(4)]
            wj2_all = consts.tile([P, 4 * V], BF16, tag="wj2")
            nc.gpsimd.dma_start(wj2_all[:], wj2[:, :])
            wj2_sb = [wj2_all[:, jc * V : (jc + 1) * V] for jc in range(4)]
            if SB:
                encb = encbp.tile([P, 4 * SBP], BF16, tag="encpB")
                nc.gpsimd.dma_start(encb[:], encpB[:, :])
                bvB_all = consts.tile([P, 4 * NB], F32, tag="bvB")
                nc.gpsimd.dma_start(bvB_all[:], bvB[:, :])
                bvB_sb = [bvB_all[:, jc * NB : (jc + 1) * NB] for jc in range(4)]

            # ---- engine warmups during the DMA-wait window ----
            # ACT: a dummy Relu triggers the ~1.3us activation-table load
            # PE: dummy matmuls ramp the HAM clock gate (cold -> full speed)
            nc.scalar.memzero(warm2[:])
            nc.scalar.activation(
                warm2[:], warm2[:], mybir.ActivationFunctionType.Relu,
                bias=0.0, scale=1.0,
            )
            for i in range(NWARM):
                wps = psj.tile([P, 2 * 512], F32, tag="psj")
                nc.tensor.matmul(
                    wps[:, : 4 * P], warm[:, :P], warm[:], start=True, stop=True
                )

            # ---- static engine balancer (measured per-op costs) ----
            loads = {"v": 0.0, "s": 0.0, "g": 0.0}
            npool = [0]
            ecost = lambda tab: {"v": tab[0], "s": tab[1], "g": tab[2]}
            CO = {
                "ha": ecost(C_HA), "hb": ecost(C_HB),
                "epa": ecost(C_EPA), "epb": ecost(C_EPB),
            }

            def pick(kind, allowed=("v", "s", "g")):
                if npool[0] >= POOL_MAX:
                    allowed = tuple(e for e in allowed if e != "g")
                e = min(allowed, key=lambda x: loads[x] + CO[kind][x])
                loads[e] += CO[kind][e]
                if e == "g":
                    npool[0] += 1
                return e

            def h_op(dst, src, bias_ap, eng):
                if eng == "v":
                    nc.vector.tensor_scalar(
                        dst, src, bias_ap, 0.0,
                        mybir.AluOpType.add, mybir.AluOpType.max,
                    )
                elif eng == "g":
                    nc.gpsimd.tensor_scalar(
                        dst, src, bias_ap, 0.0,
                        mybir.AluOpType.add, mybir.AluOpType.max,
                    )
                else:
                    nc.scalar.activation(
                        dst, src, mybir.ActivationFunctionType.Relu,
                        bias=bias_ap, scale=1.0,
                    )

            def copy_on(eng, dst, src):
                if eng == "v":
                    nc.vector.tensor_copy(dst, src)
                else:
                    nc.scalar.copy(dst, src)

            def emit_b_group(s, g):
                h4 = []
                for jc in range(4):
                    ht = hp.tile([P, 4 * P], BF16, tag=f"h{jc}")
                    for ci in range(4):
                        c = g * 4 + ci
                        idx = s * CB + c
                        h_op(
                            ht[:, ci * P : (ci + 1) * P],
                            encb[:, jc * SBP + s * P : jc * SBP + (s + 1) * P],
                            bvB_sb[jc][:, idx : idx + 1],
                            pick("hb"),
                        )
                    h4.append(ht)
                ps = psj.tile([P, 2 * 512], F32, tag="psj")
                for jc in range(4):
                    nc.tensor.matmul(
                        ps[:, : 4 * P], wj2_sb[jc], h4[jc][:],
                        start=(jc == 0), stop=(jc == 3),
                    )
                gi = s * (CB // 4) + g
                ot = outp.tile([P, 4 * P], BF16, tag="outb")
                copy_on(pick("epb", ("v", "s")), ot[:], ps[:, : 4 * P])
                nc.sync.dma_start(outB[gi], ot[:])

            # B groups interleaved among A items (spread S-engine load)
            bqueue = [(s, g) for s in range(SB) for g in range(CB // 4)] if SB else []
            n_total_items = max(NITA, 1)
            bstep = max(1, n_total_items // (len(bqueue) + 1)) if bqueue else 0

            # ---- section A: width-3 slots, paired 2-bank PSUM epilogue ----
            # epilogues are emitted LAGGED so the in-order scalar engine never
            # head-of-line blocks a later item's Relu on an unfinished chain
            ps_pair = None
            ot_pair = None
            pending = []

            NPAIR_A = NITA // 2 if PAIRED else 0

            def flush_epi():
                ppair, opair, pidx = pending.pop(0)
                if pidx >= NPAIR_A - 2:
                    # tail pairs: split halves across V and S for latency
                    nc.vector.tensor_copy(opair[:, :WAP], ppair[:, :WAP])
                    nc.scalar.copy(opair[:, WAP:], ppair[:, 512 : 512 + WAP])
                else:
                    src_v = ppair[:].rearrange("p (g x) -> p g x", g=2)[:, :, :WAP]
                    dst_v = opair[:].rearrange("p (g x) -> p g x", g=2)
                    copy_on(pick("epa", ("v", "s")), dst_v, src_v)
                nc.sync.dma_start(outA[pidx], opair[:])

            for s in range(SA):
                enc_next = None
                for c in range(CA):
                    if c == min(2, CA - 1) and s + 1 < SA:
                        enc_next = load_slot(s + 1, ring=nc.gpsimd)
                    idx = s * CA + c
                    h4 = []
                    for jc in range(4):
                        ht = hp.tile([P, WAP], BF16, tag=f"h{jc}")
                        eng = "v" if idx >= NITA - 1 else pick("ha")
                        h_op(
                            ht[:],
                            enc_cur[:, jc * WAP : (jc + 1) * WAP],
                            bvA_sb[jc][:, idx : idx + 1],
                            eng,
                        )
                        h4.append(ht)
                    if PAIRED:
                        half = idx % 2
                        if half == 0:
                            ps_pair = psj.tile([P, 2 * 512], F32, tag="psj")
                            ot_pair = outp.tile([P, 2 * WAP], BF16, tag="out")
                        dst = ps_pair[:, half * 512 : half * 512 + WAP]
                        for jc in range(4):
                            nc.tensor.matmul(
                                dst, wj2_sb[jc], h4[jc][:],
                                start=(jc == 0), stop=(jc == 3),
                            )
                        nc.tensor.matmul(
                            ps_pair[:, 896:1024], warm[:, :P], warm[:, :P],
                            start=True, stop=True,
                        )
                        if half == 1:
                            pending.append((ps_pair, ot_pair, idx // 2))
                            if len(pending) > 1:
                                flush_epi()
                    else:
                        ps = psj.tile([P, 2 * 512], F32, tag="psj")
                        for jc in range(4):
                            nc.tensor.matmul(
                                ps[:, :WAP], wj2_sb[jc], h4[jc][:],
                                start=(jc == 0), stop=(jc == 3),
                            )
                        ot = outp.tile([P, WAP], BF16, tag="out")
                        copy_on(pick("epb", ("v", "s")), ot[:], ps[:, :WAP])
                        nc.sync.dma_start(outA[idx], ot[:])
                    if bqueue and bstep and idx % bstep == bstep - 1:
                        emit_b_group(*bqueue.pop(0))
                if enc_next is not None:
                    enc_cur = enc_next

            while pending:
                flush_epi()
            while bqueue:
                emit_b_group(*bqueue.pop(0))
            if os.environ.get("KERNEL_DEBUG"):
                print(f"balancer loads: {loads} pool_ops={npool[0]}")
    _split_excess_waits(nc)
    return nc


def _host_bvec(targets, emb, W1, b1, W2, b2, Wj1, bj1):
    """Prediction network on host -> bvec[b, u, JOIN] (pred_proj + bj1)."""
    tgt = np.asarray(targets).astype(np.int64)
    ext = np.pad(tgt, ((0, 0), (H, 0)), constant_values=V - 1)  # [B, U+H]
    ctx0 = ext[:, 1 : 1 + NU]
    ctx1 = ext[:, 0:NU]
    e = np.concatenate([emb[ctx0], emb[ctx1]], axis=-1)  # [B, NU, H*EMB]
    p = np.maximum(e @ W1 + b1, 0.0)
    pred = np.maximum(p @ W2 + b2, 0.0)  # [B, NU, PRED]
    Wp = Wj1[ENC:]
    return (pred @ Wp + bj1).astype(np.float32)  # [B, NU, JOIN]


def _schedule(enc_sizes, tgt_sizes):
    """Decompose the ragged grid into width-3 / width-1 chunk work and
    LPT-pack it onto 8 cores.  Returns (SA, CA, SB, CB, cores, leftover):
    cores[i] = {"aslots": [(b,t0,w)], "agrid": [[item or None]*CA]*SA,
                "bslots": [(b,t0,w)], "bgrid": ...}; item = (b, t0, w, u);
    leftover = [(b, t0, w, u)] to compute on the host."""
    w3, w1 = [], []  # chunks: (b, t0, width, ucnt)
    for b in range(B):
        ttiles = max(1, math.ceil(int(enc_sizes[b]) / P))
        ucnt = int(tgt_sizes[b]) + 1
        t = 0
        while ttiles - t >= 3:
            w3.append((b, t * P, 3, ucnt))
            t += 3
        rem = ttiles - t
        if rem == 2:
            w3.append((b, t * P, 2, ucnt))  # padded into a width-3 slot
        elif rem == 1:
            w1.append((b, t * P, 1, ucnt))

    n3 = sum(c[3] for c in w3)
    n1 = sum(c[3] for c in w1)
    CA = 11
    CB = 4

    total_units = 3.0 * n3 + 1.0 * n1
    target = total_units / 8.0

    def pack(chunks, S, C, loads, weight):
        cores = [
            {"slots": [], "grid": [[None] * C for _ in range(S)], "items": 0}
            for _ in range(8)
        ]
        leftover = []
        for b, t0, w, n in sorted(chunks, key=lambda c: -c[3]):
            u0 = 0
            left = n
            while left > 0:
                order = sorted(range(8), key=lambda i: loads[i])
                placed = False
                for i in order:
                    cc = cores[i]
                    cap = (S - len(cc["slots"])) * C
                    if cap <= 0:
                        continue
                    # don't let one core grab far more than its fair share
                    fair = max(C, int(round((target - loads[i]) / weight / C)) * C)
                    take = min(left, cap, fair)
                    nslots = math.ceil(take / C)
                    base = len(cc["slots"])
                    for j in range(take):
                        si = base + j // C
                        cc["grid"][si][j % C] = (b, t0, w, u0 + j)
                    for _ in range(nslots):
                        cc["slots"].append((b, t0, w))
                    cc["items"] += take
                    loads[i] += take * weight
                    u0 += take
                    left -= take
                    placed = True
                    break
                if not placed:
                    for j in range(left):
                        leftover.append((b, t0, w, u0 + j))
                    break
        return cores, leftover

    # device time is proportional to grid CAPACITY (every cell is computed),
    # so try configs in increasing total-cost order and accept the first
    # whose unpacked remainder is small enough to compute on the host.
    SA0 = max(1, math.ceil((n3 / 8) / CA))
    SB0 = min(3, math.ceil((n1 / 8) / CB)) if n1 else 0
    configs = []
    for da in range(3):
        for db in range(3):
            SA_t = SA0 + da
            SB_t = min(3, SB0 + db) if n1 else 0
            cost = SA_t * CA * 3 + SB_t * CB
            configs.append((cost, SA_t, SB_t))
    configs = sorted(set(configs))
    if os.environ.get("KERNEL_FORCE_SA"):
        fsa = int(os.environ["KERNEL_FORCE_SA"])
        fsb = int(os.environ.get("KERNEL_FORCE_SB", SB0 or 0))
        configs = [(0, fsa, fsb)]
    best = None
    for cost, SA, SB in configs:
        loads = [0.0] * 8
        acores, aleft = pack(w3, SA, CA, loads, 3.0)
        if SB:
            bcores, bleft = pack(w1, SB, CB, loads, 1.0)
        else:
            bcores = [{"slots": [], "grid": [], "items": 0} for _ in range(8)]
            bleft = []
        nleft = len(aleft) + len(bleft)
        cand = (nleft, SA, SB, acores, bcores, aleft + bleft)
        if best is None or cand[0] < best[0]:
            best = cand
        if nleft <= 18:  # small host fallback is cheaper than a bigger grid
            break
    _, SA, SB, acores, bcores, leftover = best
    cores = []
    for i in range(8):
        cores.append({
            "aslots": acores[i]["slots"], "agrid": acores[i]["grid"],
            "bslots": bcores[i]["slots"], "bgrid": bcores[i]["grid"],
        })
    return SA, CA, SB, CB, cores, leftover


def _get_compiled(key):
    if key not in _CACHE:
        _CACHE[key] = _build_nc(*key)
    return _CACHE[key]


def kernel(
    encoder_states,
    encoder_states_size,
    targets,
    targets_size,
    emb,
    W1,
    b1,
    W2,
    b2,
    Wj1,
    bj1,
    Wj2,
    bj2,
):
    import ml_dtypes

    enc = np.ascontiguousarray(np.asarray(encoder_states, dtype=np.float32))
    enc_sizes = np.asarray(encoder_states_size).astype(np.int64)
    tgt_sizes = np.asarray(targets_size).astype(np.int64)
    emb = np.asarray(emb, dtype=np.float32)
    W1 = np.asarray(W1, dtype=np.float32)
    b1 = np.asarray(b1, dtype=np.float32)
    W2 = np.asarray(W2, dtype=np.float32)
    b2 = np.asarray(b2, dtype=np.float32)
    Wj1 = np.asarray(Wj1, dtype=np.float32)
    bj1 = np.asarray(bj1, dtype=np.float32)
    Wj2 = np.ascontiguousarray(np.asarray(Wj2, dtype=np.float32))
    bj2 = np.asarray(bj2, dtype=np.float32)

    bf16 = ml_dtypes.bfloat16
    bvec = _host_bvec(targets, emb, W1, b1, W2, b2, Wj1, bj1)
    We = np.ascontiguousarray(Wj1[:ENC])
    SA, CA, SB, CB, cores, leftover = _schedule(enc_sizes, tgt_sizes)
    WA = 3
    WAP = WA * P

    nc = _get_compiled((SA, CA, SB, CB))

    trace = bool(os.environ.get("KERNEL_TRACE"))
    if trace:
        _install_ntff_hook()

    # host encoder projection: encp[b] = enc[b] @ We -> transposed [4, 128, T]
    encp = np.matmul(enc, We)  # [B, T, JOIN] fp32
    encpT = np.ascontiguousarray(encp.transpose(0, 2, 1)).reshape(B, 4, P, T)
    encpT_c = encpT.astype(bf16)
    Wj2_c = Wj2.astype(bf16)
    bvec_c = bvec  # [B, NU, JOIN] fp32 (tensor_scalar needs fp32 scalar)

    NITA = SA * CA
    NA = max(NITA, 1)
    WAP = WA * P
    # pre-arranged flat layouts: every DMA is [128, free] contiguous
    wj2_flat = np.ascontiguousarray(
        Wj2_c.reshape(4, P, V).transpose(1, 0, 2).reshape(P, 4 * V)
    )
    in_maps = []
    for core in cores:
        encpA_arr = np.zeros((SA, P, 4 * WAP), dtype=bf16)
        for si, (b, t0, w) in enumerate(core["aslots"]):
            wid = w * P
            for jc in range(4):
                encpA_arr[si, :, jc * WAP : jc * WAP + wid] = encpT_c[
                    b, jc, :, t0 : t0 + wid
                ]
        bvA_arr = np.zeros((P, 4 * NA), dtype=np.float32)
        for si in range(SA):
            for c in range(CA):
                it = core["agrid"][si][c]
                if it is None:
                    continue
                b, t0, w, u = it
                bv4 = bvec[b, u].reshape(4, P)
                for jc in range(4):
                    bvA_arr[:, jc * NA + si * CA + c] = bv4[jc]
        m = {
            "encpA": encpA_arr,
            "bvA": bvA_arr,
            "wj2": wj2_flat,
        }
        if SB:
            SBP = SB * P
            NB = SB * CB
            encpB_arr = np.zeros((P, 4 * SBP), dtype=bf16)
            for si, (b, t0, w) in enumerate(core["bslots"]):
                for jc in range(4):
                    encpB_arr[:, jc * SBP + si * P : jc * SBP + (si + 1) * P] = (
                        encpT_c[b, jc, :, t0 : t0 + P]
                    )
            bvB_arr = np.zeros((P, 4 * NB), dtype=np.float32)
            for si in range(SB):
                for c in range(CB):
                    it = core["bgrid"][si][c]
                    if it is None:
                        continue
                    b, t0, w, u = it
                    bv4 = bvec[b, u].reshape(4, P)
                    for jc in range(4):
                        bvB_arr[:, jc * NB + si * CB + c] = bv4[jc]
            m["encpB"] = encpB_arr
            m["bvB"] = bvB_arr
        in_maps.append(m)

    kwargs = {}
    if trace:
        kwargs = dict(trace=True, trace_cores=list(range(8)))
    res = None
    last_exc = None
    for attempt in range(3):
        try:
            res = bass_utils.run_bass_kernel_spmd(
                nc, in_maps, core_ids=list(range(8)), **kwargs
            )
            break
        except Exception as e:  # transient device wedges happen; retry
            last_exc = e
            import time as _time

            _time.sleep(2.0)
    if res is None:
        raise last_exc
    kernel.last_results = [res]

    final = np.zeros((B, T, NU, V), dtype=np.float32)
    for ki, core in enumerate(cores):
        outA = np.asarray(res.results[ki]["outA"]).astype(np.float32)
        if NITA % 2 == 0 and NITA > 0:
            outA = outA.reshape(NITA // 2, P, 2, WA * P).transpose(0, 2, 1, 3).reshape(
                NITA, P, WA * P
            )
        for si in range(SA):
            for c in range(CA):
                it = core["agrid"][si][c]
                if it is None:
                    continue
                b, t0, w, u = it
                rows = min(w * P, int(enc_sizes[b]) - t0)
                if rows <= 0:
                    continue
                final[b, t0 : t0 + rows, u, :] = outA[si * CA + c, :, :rows].T + bj2
        if SB:
            outB = np.asarray(res.results[ki]["outB"]).astype(np.float32)
            for si in range(SB):
                for c in range(CB):
                    it = core["bgrid"][si][c]
                    if it is None:
                        continue
                    b, t0, w, u = it
                    rows = min(P, int(enc_sizes[b]) - t0)
                    if rows <= 0:
                        continue
                    gi = si * (CB // 4) + c // 4
                    ci = c % 4
                    final[b, t0 : t0 + rows, u, :] = (
                        outB[gi, :, ci * P : ci * P + rows].T + bj2
                    )

    # host fallback for anything that didn't fit the device grids
    if leftover:
        bychunk = {}
        for b, t0, w, u in leftover:
            bychunk.setdefault((b, t0, w), []).append(u)
        for (b, t0, w), us in bychunk.items():
            rows = min(w * P, int(enc_sizes[b]) - t0)
            if rows <= 0:
                continue
            ep = encp[b, t0 : t0 + rows, :]  # [rows, JOIN] fp32
            for u in us:
                hh = np.maximum(ep + bvec[b, u], 0.0)
                final[b, t0 : t0 + rows, u, :] = hh @ Wj2 + bj2

    return final



# revision 30
# speedup vs baseline: 1.4966x; 1.4966x over previous
"""Trainium2 Bass kernel for nn_FFNNTransducerModel (RNN-T style transducer).

Strategy (v2)
-------------
The output grid [B, T, U+1, V] is ragged: only t < enc_size[b], u <= tgt_size[b]
is nonzero (the reference multiplies by that mask).

  host:   - prediction network (embedding + 2-layer MLP + Wp proj + bj1)
            -> per-(b,u) bias vector bvec[b,u,512]        (tiny)
          - encoder projection encp = enc @ We  -> [B, T, 512]  (one GEMM)
          - decompose each example's valid t-tiles into width-3 and width-1
            tile chunks, LPT-pack (chunk, u) items across the 8 cores into
            two fixed grids (SPMD: one program, per-core data)
  device: - per item: h[jc] = relu(encp_T[jc, t-range] + bvec[u])  (DVE/ACT)
          - joint GEMM: psum[v, t*] += wj2[jc].T @ h[jc]  (fp32 PSUM accum)
          - epilogue: copy psum -> SBUF bf16, DMA out  (bj2 added on host)
  host:   - out.astype(f32) + bj2, scatter item tiles (transposed) into the
            zero-initialized output; invalid region stays exactly 0.

All device tensors are bf16 (fp32 PSUM accumulation).  The compiled program
depends only on the grid shape, which is derived from input sizes and cached.
"""

import math
import os
import sys
import types

import numpy as np

import concourse.bass as bass
import concourse.mybir as mybir
import concourse.tile as tile
from concourse import bass_utils

F32 = mybir.dt.float32
BF16 = mybir.dt.bfloat16
P = 128

# Model dims (fixed by the problem)
B, T, U, V = 8, 512, 64, 128
ENC, PRED, JOIN, EMB, H = 512, 256, 512, 128, 2
NU = U + 1  # 65

NWARM = int(os.environ.get("KERNEL_NWARM", "6"))
# measured per-op engine costs (ns) used by the static balancer: v, s, g
C_HA = [float(x) for x in os.environ.get("KERNEL_C_HA", "375,465,1600").split(",")]
C_HB = [float(x) for x in os.environ.get("KERNEL_C_HB", "295,380,900").split(",")]
C_EPA = [float(x) for x in os.environ.get("KERNEL_C_EPA", "1085,1070,1e9").split(",")]
C_EPB = [float(x) for x in os.environ.get("KERNEL_C_EPB", "790,823,1e9").split(",")]
# Pool (gpsimd) h-ops: measured ~5.7us per [128,384] op — keep at 0
POOL_MAX = int(os.environ.get("KERNEL_POOL_MAX", "0"))

_CACHE = {}


def _install_ntff_hook():
    """The image's antenv lacks axon_hooks; shim it so trace=True works."""
    if "antenv.axon_hooks" in sys.modules:
        return
    mod = types.ModuleType("antenv.axon_hooks")
    _hook = [None]
    mod.set_axon_ntff_profile_hook = lambda h: _hook.__setitem__(0, h)
    mod.get_axon_ntff_profile_hook = lambda: _hook[0]
    sys.modules["antenv.axon_hooks"] = mod
    try:
        from trn_agent_boot.trn_boot import _ntff_profile_via_ctypes

        mod.set_axon_ntff_profile_hook(
            _ntff_profile_via_ctypes("/opt/axon/libaxon_pjrt.so")
        )
    except Exception:
        pass


def _split_excess_waits(nc, max_waits=1):
    """This container's walrus supports only one embedded sync-wait per
    instruction; split extras into standalone EventSemaphore waits placed
    immediately before the consumer on the same engine stream."""
    f = nc.m.functions[0]
    for blk in f.blocks:
        insts = list(blk.instructions)
        out = []
        changed = False
        for ins in insts:
            si = getattr(ins, "sync_info", None)
            if si is not None and si.on_wait is not None and len(si.on_wait) > max_waits:
                waits = list(si.on_wait)
                keep, excess = waits[:max_waits], waits[max_waits:]
                for j, w in enumerate(excess):
                    es = mybir.InstEventSemaphore(
                        name=f"{ins.name}_xw{j}",
                        engine=ins.engine,
                        sync_info=mybir.SyncInfo(on_wait=[w], on_update=[]),
                    )
                    out.append(es)
                si.on_wait = keep
                changed = True
            out.append(ins)
        if changed:
            blk.instructions = out
    return nc


def _build_nc(SA, CA, SB, CB):
    """Uniform SPMD program; all data dependence lives in the input arrays.

    Section A: SA slots of width WA=3 t-tiles, CA items (u values) each.
    Section B: SB slots of width 1 t-tile, CB items each, grouped by 4
    into one PSUM bank (CB % 4 == 0).  B groups are interleaved among the
    A items so scalar-engine work doesn't pile up at the end.

    All DRAM inputs are pre-arranged on the host into the exact SBUF
    layout ([128, free]) so every DMA is one descriptor per partition."""
    WA = 3
    WAP = WA * P
    NITA = SA * CA
    NA = max(NITA, 1)
    NGB = (SB * CB) // 4 if SB else 0
    PAIRED = NITA > 0 and NITA % 2 == 0
    NPAIR = NA // 2 if PAIRED else NA

    nc = bass.Bass()
    encpA = nc.dram_tensor("encpA", [SA, P, 4 * WAP], BF16, kind="ExternalInput")
    bvA = nc.dram_tensor("bvA", [P, 4 * NA], F32, kind="ExternalInput")
    if SB:
        # B section: enc replicated per item and a broadcast bias tile, so ONE
        # big tensor_tensor(max) covers a whole 4-item group (max-route; the
        # host adds the bvec@Wj2 correction for B items during the scatter)
        encpB = nc.dram_tensor("encpB", [SB, P, 2048], BF16, kind="ExternalInput")
        nbtB = nc.dram_tensor("nbtB", [SB, P, 2048], BF16, kind="ExternalInput")
    wj2 = nc.dram_tensor("wj2", [P, 4 * V], BF16, kind="ExternalInput")
    outA = nc.dram_tensor(
        "outA", [NPAIR, P, (2 if PAIRED else 1) * WAP], BF16, kind="ExternalOutput"
    )
    if SB:
        outB = nc.dram_tensor("outB", [NGB, P, 4 * P], BF16, kind="ExternalOutput")

    with tile.TileContext(nc) as tc:
        with (
            tc.tile_pool(name="consts", bufs=1) as consts,
            tc.tile_pool(name="encpp", bufs=2) as encpp,
            tc.tile_pool(name="encbp", bufs=1) as encbp,
            tc.tile_pool(name="hp", bufs=12) as hp,
            tc.tile_pool(name="outp", bufs=5) as outp,
            tc.tile_pool(name="psj", bufs=4, space="PSUM") as psj,
        ):
            # warm tile init on Pool — its queue is free earliest, and the
            # PE warmup matmuls can then start during the DMA-wait window
            warm = consts.tile([P, 4 * P], BF16, tag="warm")
            nc.gpsimd.memset(warm[:], 0.0)
            warm2 = consts.tile([P, P], BF16, tag="warm2")

            def load_slot(s, split=False, ring=None):
                t = encpp.tile([P, 4 * WAP], BF16, tag="encp")
                dma = (ring or nc.sync).dma_start
                if split:
                    dma(t[:, : 2 * WAP], encpA[s][:, : 2 * WAP])
                    dma(t[:, 2 * WAP :], encpA[s][:, 2 * WAP :])
                else:
                    dma(t[:], encpA[s])
                return t

            # critical first loads: slot0 on the SP ring, the small bias
            # array first on the ACT ring (lands earliest), rest on Pool ring
            bvA_all = consts.tile([P, 4 * NA], F32, tag="bvA")
            nc.scalar.dma_start(bvA_all[:], bvA[:, :])
            enc_cur = load_slot(0, split=True) if SA else None
            bvA_sb = [bvA_all[:, jc * NA : (jc + 1) * NA] for jc in range(4)]
            wj2_all = consts.tile([P, 4 * V], BF16, tag="wj2")
            nc.gpsimd.dma_start(wj2_all[:], wj2[:, :])
            wj2_sb = [wj2_all[:, jc * V : (jc + 1) * V] for jc in range(4)]
            if SB:
                encb = encbp.tile([P, SB * 2048], BF16, tag="encpB")
                nbtb = encbp.tile([P, SB * 2048], BF16, tag="nbtB")
                for s in range(SB):
                    nc.gpsimd.dma_start(encb[:, s * 2048 : (s + 1) * 2048], encpB[s])
                    nc.gpsimd.dma_start(nbtb[:, s * 2048 : (s + 1) * 2048], nbtB[s])

            # ---- engine warmups during the DMA-wait window ----
            # ACT: a dummy Relu triggers the ~1.3us activation-table load
            # PE: dummy matmuls ramp the HAM clock gate (cold -> full speed)
            nc.scalar.memzero(warm2[:])
            nc.scalar.activation(
                warm2[:], warm2[:], mybir.ActivationFunctionType.Relu,
                bias=0.0, scale=1.0,
            )
            for i in range(NWARM):
                wps = psj.tile([P, 2 * 512], F32, tag="psj")
                nc.tensor.matmul(
                    wps[:, : 4 * P], warm[:, :P], warm[:], start=True, stop=True
                )

            # ---- static engine balancer (measured per-op costs) ----
            loads = {"v": 0.0, "s": 0.0, "g": 0.0}
            npool = [0]
            ecost = lambda tab: {"v": tab[0], "s": tab[1], "g": tab[2]}
            CO = {
                "ha": ecost(C_HA), "hb": ecost(C_HB),
                "epa": ecost(C_EPA), "epb": ecost(C_EPB),
            }

            def pick(kind, allowed=("v", "s", "g")):
                if npool[0] >= POOL_MAX:
                    allowed = tuple(e for e in allowed if e != "g")
                e = min(allowed, key=lambda x: loads[x] + CO[kind][x])
                loads[e] += CO[kind][e]
                if e == "g":
                    npool[0] += 1
                return e

            def h_op(dst, src, bias_ap, eng):
                if eng == "v":
                    nc.vector.tensor_scalar(
                        dst, src, bias_ap, 0.0,
                        mybir.AluOpType.add, mybir.AluOpType.max,
                    )
                elif eng == "g":
                    nc.gpsimd.tensor_scalar(
                        dst, src, bias_ap, 0.0,
                        mybir.AluOpType.add, mybir.AluOpType.max,
                    )
                else:
                    nc.scalar.activation(
                        dst, src, mybir.ActivationFunctionType.Relu,
                        bias=bias_ap, scale=1.0,
                    )

            def copy_on(eng, dst, src):
                if eng == "v":
                    nc.vector.tensor_copy(dst, src)
                else:
                    nc.scalar.copy(dst, src)

            def emit_b_group(s, g):
                # one fused max over the whole 4-item group: layout
                # [jc][ci][t] with enc replicated per ci on the host
                ht = hp.tile([P, 2048], BF16, tag="hb")
                loads["v"] += 1320.0
                nc.vector.tensor_tensor(
                    ht[:],
                    encb[:, s * 2048 : (s + 1) * 2048],
                    nbtb[:, s * 2048 : (s + 1) * 2048],
                    mybir.AluOpType.max,
                )
                ps = psj.tile([P, 2 * 512], F32, tag="psj")
                for jc in range(4):
                    nc.tensor.matmul(
                        ps[:, : 4 * P], wj2_sb[jc],
                        ht[:, jc * 512 : (jc + 1) * 512],
                        start=(jc == 0), stop=(jc == 3),
                    )
                gi = s * (CB // 4) + g
                ot = outp.tile([P, 4 * P], BF16, tag="outb")
                copy_on(pick("epb", ("v", "s")), ot[:], ps[:, : 4 * P])
                nc.sync.dma_start(outB[gi], ot[:])

            # B groups interleaved among A items (spread S-engine load)
            bqueue = [(s, g) for s in range(SB) for g in range(CB // 4)] if SB else []
            n_total_items = max(NITA, 1)
            bstep = max(1, n_total_items // (len(bqueue) + 1)) if bqueue else 0

            # ---- section A: width-3 slots, paired 2-bank PSUM epilogue ----
            # epilogues are emitted LAGGED so the in-order scalar engine never
            # head-of-line blocks a later item's Relu on an unfinished chain
            ps_pair = None
            ot_pair = None
            pending = []

            NPAIR_A = NITA // 2 if PAIRED else 0

            def flush_epi():
                ppair, opair, pidx = pending.pop(0)
                if pidx >= NPAIR_A - 2:
                    # tail pairs: split halves across V and S, and DMA each
                    # half as soon as its copy lands (latency, not throughput)
                    nc.vector.tensor_copy(opair[:, :WAP], ppair[:, :WAP])
                    nc.sync.dma_start(outA[pidx][:, :WAP], opair[:, :WAP])
                    nc.scalar.copy(opair[:, WAP:], ppair[:, 512 : 512 + WAP])
                    nc.sync.dma_start(outA[pidx][:, WAP:], opair[:, WAP:])
                else:
                    src_v = ppair[:].rearrange("p (g x) -> p g x", g=2)[:, :, :WAP]
                    dst_v = opair[:].rearrange("p (g x) -> p g x", g=2)
                    copy_on(pick("epa", ("v", "s")), dst_v, src_v)
                    nc.sync.dma_start(outA[pidx], opair[:])

            for s in range(SA):
                enc_next = None
                for c in range(CA):
                    if c == min(2, CA - 1) and s + 1 < SA:
                        enc_next = load_slot(s + 1, ring=nc.gpsimd)
                    idx = s * CA + c
                    h4 = []
                    for jc in range(4):
                        ht = hp.tile([P, WAP], BF16, tag=f"h{jc}")
                        eng = "v" if idx >= NITA - 1 else pick("ha")
                        h_op(
                            ht[:],
                            enc_cur[:, jc * WAP : (jc + 1) * WAP],
                            bvA_sb[jc][:, idx : idx + 1],
                            eng,
                        )
                        h4.append(ht)
                    if PAIRED:
                        half = idx % 2
                        if half == 0:
                            ps_pair = psj.tile([P, 2 * 512], F32, tag="psj")
                            ot_pair = outp.tile([P, 2 * WAP], BF16, tag="out")
                        dst = ps_pair[:, half * 512 : half * 512 + WAP]
                        for jc in range(4):
                            nc.tensor.matmul(
                                dst, wj2_sb[jc], h4[jc][:],
                                start=(jc == 0), stop=(jc == 3),
                            )
                        nc.tensor.matmul(
                            ps_pair[:, 896:1024], warm[:, :P], warm[:, :P],
                            start=True, stop=True,
                        )
                        if half == 1:
                            pending.append((ps_pair, ot_pair, idx // 2))
                            if len(pending) > 1:
                                flush_epi()
                    else:
                        ps = psj.tile([P, 2 * 512], F32, tag="psj")
                        for jc in range(4):
                            nc.tensor.matmul(
                                ps[:, :WAP], wj2_sb[jc], h4[jc][:],
                                start=(jc == 0), stop=(jc == 3),
                            )
                        ot = outp.tile([P, WAP], BF16, tag="out")
                        copy_on(pick("epb", ("v", "s")), ot[:], ps[:, :WAP])
                        nc.sync.dma_start(outA[idx], ot[:])
                    if bqueue and bstep and idx % bstep == bstep - 1:
                        emit_b_group(*bqueue.pop(0))
                if enc_next is not None:
                    enc_cur = enc_next

            while pending:
                flush_epi()
            while bqueue:
                emit_b_group(*bqueue.pop(0))
            if os.environ.get("KERNEL_DEBUG"):
                print(f"balancer loads: {loads} pool_ops={npool[0]}")
    _split_excess_waits(nc)
    return nc


def _host_bvec(targets, emb, W1, b1, W2, b2, Wj1, bj1):
    """Prediction network on host -> bvec[b, u, JOIN] (pred_proj + bj1)."""
    tgt = np.asarray(targets).astype(np.int64)
    ext = np.pad(tgt, ((0, 0), (H, 0)), constant_values=V - 1)  # [B, U+H]
    ctx0 = ext[:, 1 : 1 + NU]
    ctx1 = ext[:, 0:NU]
    e = np.concatenate([emb[ctx0], emb[ctx1]], axis=-1)  # [B, NU, H*EMB]
    p = np.maximum(e @ W1 + b1, 0.0)
    pred = np.maximum(p @ W2 + b2, 0.0)  # [B, NU, PRED]
    Wp = Wj1[ENC:]
    return (pred @ Wp + bj1).astype(np.float32)  # [B, NU, JOIN]


def _schedule(enc_sizes, tgt_sizes):
    """Decompose the ragged grid into width-3 / width-1 chunk work and
    LPT-pack it onto 8 cores.  Returns (SA, CA, SB, CB, cores, leftover):
    cores[i] = {"aslots": [(b,t0,w)], "agrid": [[item or None]*CA]*SA,
                "bslots": [(b,t0,w)], "bgrid": ...}; item = (b, t0, w, u);
    leftover = [(b, t0, w, u)] to compute on the host."""
    w3, w1 = [], []  # chunks: (b, t0, width, ucnt)
    for b in range(B):
        ttiles = max(1, math.ceil(int(enc_sizes[b]) / P))
        ucnt = int(tgt_sizes[b]) + 1
        t = 0
        while ttiles - t >= 3:
            w3.append((b, t * P, 3, ucnt))
            t += 3
        rem = ttiles - t
        if rem == 2:
            w3.append((b, t * P, 2, ucnt))  # padded into a width-3 slot
        elif rem == 1:
            w1.append((b, t * P, 1, ucnt))

    n3 = sum(c[3] for c in w3)
    n1 = sum(c[3] for c in w1)
    CA = 11
    CB = 4

    total_units = 3.0 * n3 + 1.0 * n1
    target = total_units / 8.0

    def pack(chunks, S, C, loads, weight):
        cores = [
            {"slots": [], "grid": [[None] * C for _ in range(S)], "items": 0}
            for _ in range(8)
        ]
        leftover = []
        for b, t0, w, n in sorted(chunks, key=lambda c: -c[3]):
            u0 = 0
            left = n
            while left > 0:
                order = sorted(range(8), key=lambda i: loads[i])
                placed = False
                for i in order:
                    cc = cores[i]
                    cap = (S - len(cc["slots"])) * C
                    if cap <= 0:
                        continue
                    # don't let one core grab far more than its fair share
                    fair = max(C, int(round((target - loads[i]) / weight / C)) * C)
                    take = min(left, cap, fair)
                    nslots = math.ceil(take / C)
                    base = len(cc["slots"])
                    for j in range(take):
                        si = base + j // C
                        cc["grid"][si][j % C] = (b, t0, w, u0 + j)
                    for _ in range(nslots):
                        cc["slots"].append((b, t0, w))
                    cc["items"] += take
                    loads[i] += take * weight
                    u0 += take
                    left -= take
                    placed = True
                    break
                if not placed:
                    for j in range(left):
                        leftover.append((b, t0, w, u0 + j))
                    break
        return cores, leftover

    # device time is proportional to grid CAPACITY (every cell is computed),
    # so try configs in increasing total-cost order and accept the first
    # whose unpacked remainder is small enough to compute on the host.
    SA0 = max(1, math.ceil((n3 / 8) / CA))
    SB0 = min(3, math.ceil((n1 / 8) / CB)) if n1 else 0
    configs = []
    for da in range(3):
        for db in range(3):
            SA_t = SA0 + da
            SB_t = min(3, SB0 + db) if n1 else 0
            cost = SA_t * CA * 3 + SB_t * CB
            configs.append((cost, SA_t, SB_t))
    configs = sorted(set(configs))
    if os.environ.get("KERNEL_FORCE_SA"):
        fsa = int(os.environ["KERNEL_FORCE_SA"])
        fsb = int(os.environ.get("KERNEL_FORCE_SB", SB0 or 0))
        configs = [(0, fsa, fsb)]
    best = None
    for cost, SA, SB in configs:
        loads = [0.0] * 8
        acores, aleft = pack(w3, SA, CA, loads, 3.0)
        if SB:
            bcores, bleft = pack(w1, SB, CB, loads, 1.0)
        else:
            bcores = [{"slots": [], "grid": [], "items": 0} for _ in range(8)]
            bleft = []
        nleft = len(aleft) + len(bleft)
        cand = (nleft, SA, SB, acores, bcores, aleft + bleft)
        if best is None or cand[0] < best[0]:
            best = cand
        if nleft <= 18:  # small host fallback is cheaper than a bigger grid
            break
    _, SA, SB, acores, bcores, leftover = best
    cores = []
    for i in range(8):
        cores.append({
            "aslots": acores[i]["slots"], "agrid": acores[i]["grid"],
            "bslots": bcores[i]["slots"], "bgrid": bcores[i]["grid"],
        })
    return SA, CA, SB, CB, cores, leftover


def _get_compiled(key):
    if key not in _CACHE:
        _CACHE[key] = _build_nc(*key)
    return _CACHE[key]


def kernel(
    encoder_states,
    encoder_states_size,
    targets,
    targets_size,
    emb,
    W1,
    b1,
    W2,
    b2,
    Wj1,
    bj1,
    Wj2,
    bj2,
):
    import ml_dtypes

    enc = np.ascontiguousarray(np.asarray(encoder_states, dtype=np.float32))
    enc_sizes = np.asarray(encoder_states_size).astype(np.int64)
    tgt_sizes = np.asarray(targets_size).astype(np.int64)
    emb = np.asarray(emb, dtype=np.float32)
    W1 = np.asarray(W1, dtype=np.float32)
    b1 = np.asarray(b1, dtype=np.float32)
    W2 = np.asarray(W2, dtype=np.float32)
    b2 = np.asarray(b2, dtype=np.float32)
    Wj1 = np.asarray(Wj1, dtype=np.float32)
    bj1 = np.asarray(bj1, dtype=np.float32)
    Wj2 = np.ascontiguousarray(np.asarray(Wj2, dtype=np.float32))
    bj2 = np.asarray(bj2, dtype=np.float32)

    bf16 = ml_dtypes.bfloat16
    bvec = _host_bvec(targets, emb, W1, b1, W2, b2, Wj1, bj1)
    We = np.ascontiguousarray(Wj1[:ENC])
    SA, CA, SB, CB, cores, leftover = _schedule(enc_sizes, tgt_sizes)
    WA = 3
    WAP = WA * P

    nc = _get_compiled((SA, CA, SB, CB))

    trace = bool(os.environ.get("KERNEL_TRACE"))
    if trace:
        _install_ntff_hook()

    # host encoder projection: encp[b] = enc[b] @ We -> transposed [4, 128, T]
    encp = np.matmul(enc, We)  # [B, T, JOIN] fp32
    encpT = np.ascontiguousarray(encp.transpose(0, 2, 1)).reshape(B, 4, P, T)
    encpT_c = encpT.astype(bf16)
    Wj2_c = Wj2.astype(bf16)
    bvec_c = bvec  # [B, NU, JOIN] fp32 (tensor_scalar needs fp32 scalar)
    # B section max-route correction: corr[b,u,:] = bf16(bvec) @ bf16(Wj2)
    bvec_b16 = bvec.astype(bf16)
    corr = np.matmul(bvec_b16.astype(np.float32), Wj2_c.astype(np.float32))

    NITA = SA * CA
    NA = max(NITA, 1)
    WAP = WA * P
    # pre-arranged flat layouts: every DMA is [128, free] contiguous
    wj2_flat = np.ascontiguousarray(
        Wj2_c.reshape(4, P, V).transpose(1, 0, 2).reshape(P, 4 * V)
    )
    in_maps = []
    for core in cores:
        encpA_arr = np.zeros((SA, P, 4 * WAP), dtype=bf16)
        for si, (b, t0, w) in enumerate(core["aslots"]):
            wid = w * P
            for jc in range(4):
                encpA_arr[si, :, jc * WAP : jc * WAP + wid] = encpT_c[
                    b, jc, :, t0 : t0 + wid
                ]
        bvA_arr = np.zeros((P, 4 * NA), dtype=np.float32)
        for si in range(SA):
            for c in range(CA):
                it = core["agrid"][si][c]
                if it is None:
                    continue
                b, t0, w, u = it
                bv4 = bvec[b, u].reshape(4, P)
                for jc in range(4):
                    bvA_arr[:, jc * NA + si * CA + c] = bv4[jc]
        m = {
            "encpA": encpA_arr,
            "bvA": bvA_arr,
            "wj2": wj2_flat,
        }
        if SB:
            # layout per slot: [jc][ci][t]; enc replicated across ci, bias
            # (-bf16(bvec)) broadcast across t
            encpB_arr = np.zeros((SB, P, 2048), dtype=bf16)
            nbtB_arr = np.zeros((SB, P, 2048), dtype=bf16)
            for si, (b, t0, w) in enumerate(core["bslots"]):
                for jc in range(4):
                    blk = encpT_c[b, jc, :, t0 : t0 + P]  # [P, 128]
                    for ci in range(4):
                        encpB_arr[si, :, jc * 512 + ci * P : jc * 512 + (ci + 1) * P] = blk
            for si in range(SB):
                for c in range(CB):
                    it = core["bgrid"][si][c]
                    if it is None:
                        continue
                    b, t0, w, u = it
                    nb4 = -bvec_b16[b, u].reshape(4, P)  # [jc, p]
                    for jc in range(4):
                        nbtB_arr[si, :, jc * 512 + c * P : jc * 512 + (c + 1) * P] = (
                            nb4[jc][:, None]
                        )
            m["encpB"] = encpB_arr
            m["nbtB"] = nbtB_arr
        in_maps.append(m)

    kwargs = {}
    if trace:
        kwargs = dict(trace=True, trace_cores=list(range(8)))
    res = None
    last_exc = None
    for attempt in range(3):
        try:
            res = bass_utils.run_bass_kernel_spmd(
                nc, in_maps, core_ids=list(range(8)), **kwargs
            )
            break
        except Exception as e:  # transient device wedges happen; retry
            last_exc = e
            import time as _time

            _time.sleep(2.0)
    if res is None:
        raise last_exc
    kernel.last_results = [res]

    final = np.zeros((B, T, NU, V), dtype=np.float32)
    for ki, core in enumerate(cores):
        outA = np.asarray(res.results[ki]["outA"]).astype(np.float32)
        if NITA % 2 == 0 and NITA > 0:
            outA = outA.reshape(NITA // 2, P, 2, WA * P).transpose(0, 2, 1, 3).reshape(
                NITA, P, WA * P
            )
        for si in range(SA):
            for c in range(CA):
                it = core["agrid"][si][c]
                if it is None:
                    continue
                b, t0, w, u = it
                rows = min(w * P, int(enc_sizes[b]) - t0)
                if rows <= 0:
                    continue
                final[b, t0 : t0 + rows, u, :] = outA[si * CA + c, :, :rows].T + bj2
        if SB:
            outB = np.asarray(res.results[ki]["outB"]).astype(np.float32)
            for si in range(SB):
                for c in range(CB):
                    it = core["bgrid"][si][c]
                    if it is None:
                        continue
                    b, t0, w, u = it
                    rows = min(P, int(enc_sizes[b]) - t0)
                    if rows <= 0:
                        continue
                    gi = si * (CB // 4) + c // 4
                    ci = c % 4
                    final[b, t0 : t0 + rows, u, :] = (
                        outB[gi, :, ci * P : ci * P + rows].T + (bj2 + corr[b, u])
                    )

    # host fallback for anything that didn't fit the device grids
    if leftover:
        bychunk = {}
        for b, t0, w, u in leftover:
            bychunk.setdefault((b, t0, w), []).append(u)
        for (b, t0, w), us in bychunk.items():
            rows = min(w * P, int(enc_sizes[b]) - t0)
            if rows <= 0:
                continue
            ep = encp[b, t0 : t0 + rows, :]  # [rows, JOIN] fp32
            for u in us:
                hh = np.maximum(ep + bvec[b, u], 0.0)
                final[b, t0 : t0 + rows, u, :] = hh @ Wj2 + bj2

    return final



# revision 35
# speedup vs baseline: 1.5091x; 1.0084x over previous
"""Trainium2 Bass kernel for nn_FFNNTransducerModel (RNN-T style transducer).

Strategy (v2)
-------------
The output grid [B, T, U+1, V] is ragged: only t < enc_size[b], u <= tgt_size[b]
is nonzero (the reference multiplies by that mask).

  host:   - prediction network (embedding + 2-layer MLP + Wp proj + bj1)
            -> per-(b,u) bias vector bvec[b,u,512]        (tiny)
          - encoder projection encp = enc @ We  -> [B, T, 512]  (one GEMM)
          - decompose each example's valid t-tiles into width-3 and width-1
            tile chunks, LPT-pack (chunk, u) items across the 8 cores into
            two fixed grids (SPMD: one program, per-core data)
  device: - per item: h[jc] = relu(encp_T[jc, t-range] + bvec[u])  (DVE/ACT)
          - joint GEMM: psum[v, t*] += wj2[jc].T @ h[jc]  (fp32 PSUM accum)
          - epilogue: copy psum -> SBUF bf16, DMA out  (bj2 added on host)
  host:   - out.astype(f32) + bj2, scatter item tiles (transposed) into the
            zero-initialized output; invalid region stays exactly 0.

All device tensors are bf16 (fp32 PSUM accumulation).  The compiled program
depends only on the grid shape, which is derived from input sizes and cached.
"""

import math
import os
import sys
import types

import numpy as np

import concourse.bass as bass
import concourse.mybir as mybir
import concourse.tile as tile
from concourse import bass_utils

F32 = mybir.dt.float32
BF16 = mybir.dt.bfloat16
P = 128

# Model dims (fixed by the problem)
B, T, U, V = 8, 512, 64, 128
ENC, PRED, JOIN, EMB, H = 512, 256, 512, 128, 2
NU = U + 1  # 65

NWARM = int(os.environ.get("KERNEL_NWARM", "6"))
# measured per-op engine costs (ns) used by the static balancer: v, s, g
C_HA = [float(x) for x in os.environ.get("KERNEL_C_HA", "250,550,99999").split(",")]
C_HB = [float(x) for x in os.environ.get("KERNEL_C_HB", "295,380,99999").split(",")]
C_EPA = [float(x) for x in os.environ.get("KERNEL_C_EPA", "820,1050,1e9").split(",")]
C_EPB = [float(x) for x in os.environ.get("KERNEL_C_EPB", "700,850,1e9").split(",")]
PAIR_DUMMY = bool(int(os.environ.get("KERNEL_PAIR_DUMMY", "0")))
# Pool (gpsimd) h-ops: measured ~5.7us per [128,384] op — keep at 0
POOL_MAX = int(os.environ.get("KERNEL_POOL_MAX", "0"))

_CACHE = {}


def _install_ntff_hook():
    """The image's antenv lacks axon_hooks; shim it so trace=True works."""
    if "antenv.axon_hooks" in sys.modules:
        return
    mod = types.ModuleType("antenv.axon_hooks")
    _hook = [None]
    mod.set_axon_ntff_profile_hook = lambda h: _hook.__setitem__(0, h)
    mod.get_axon_ntff_profile_hook = lambda: _hook[0]
    sys.modules["antenv.axon_hooks"] = mod
    try:
        from trn_agent_boot.trn_boot import _ntff_profile_via_ctypes

        mod.set_axon_ntff_profile_hook(
            _ntff_profile_via_ctypes("/opt/axon/libaxon_pjrt.so")
        )
    except Exception:
        pass


def _split_excess_waits(nc, max_waits=1):
    """This container's walrus supports only one embedded sync-wait per
    instruction; split extras into standalone EventSemaphore waits placed
    immediately before the consumer on the same engine stream."""
    f = nc.m.functions[0]
    for blk in f.blocks:
        insts = list(blk.instructions)
        out = []
        changed = False
        for ins in insts:
            si = getattr(ins, "sync_info", None)
            if si is not None and si.on_wait is not None and len(si.on_wait) > max_waits:
                waits = list(si.on_wait)
                keep, excess = waits[:max_waits], waits[max_waits:]
                for j, w in enumerate(excess):
                    es = mybir.InstEventSemaphore(
                        name=f"{ins.name}_xw{j}",
                        engine=ins.engine,
                        sync_info=mybir.SyncInfo(on_wait=[w], on_update=[]),
                    )
                    out.append(es)
                si.on_wait = keep
                changed = True
            out.append(ins)
        if changed:
            blk.instructions = out
    return nc


def _build_nc(SA, CA, SB, CB):
    """Uniform SPMD program; all data dependence lives in the input arrays.

    Section A: SA slots of width WA=3 t-tiles, CA items (u values) each.
    Section B: SB slots of width 1 t-tile, CB items each, grouped by 4
    into one PSUM bank (CB % 4 == 0).  B groups are interleaved among the
    A items so scalar-engine work doesn't pile up at the end.

    All DRAM inputs are pre-arranged on the host into the exact SBUF
    layout ([128, free]) so every DMA is one descriptor per partition."""
    WA = 3
    WAP = WA * P
    NITA = SA * CA
    NA = max(NITA, 1)
    NGB = (SB * CB) // 4 if SB else 0
    PAIRED = NITA > 0 and NITA % 2 == 0
    NPAIR = NA // 2 if PAIRED else NA

    nc = bass.Bass()
    encpA = nc.dram_tensor("encpA", [SA, P, 4 * WAP], BF16, kind="ExternalInput")
    bvA = nc.dram_tensor("bvA", [P, 4 * NA], F32, kind="ExternalInput")
    if SB:
        # B section: enc replicated per item and a broadcast bias tile, so ONE
        # big tensor_tensor(max) covers a whole 4-item group (max-route; the
        # host adds the bvec@Wj2 correction for B items during the scatter)
        encpB = nc.dram_tensor("encpB", [SB, P, 2048], BF16, kind="ExternalInput")
        nbtB = nc.dram_tensor("nbtB", [SB, P, 2048], BF16, kind="ExternalInput")
    wj2 = nc.dram_tensor("wj2", [P, 4 * V], BF16, kind="ExternalInput")
    outA = nc.dram_tensor(
        "outA", [NPAIR, P, (2 if PAIRED else 1) * WAP], BF16, kind="ExternalOutput"
    )
    if SB:
        outB = nc.dram_tensor("outB", [NGB, P, 4 * P], BF16, kind="ExternalOutput")

    with tile.TileContext(nc) as tc:
        with (
            tc.tile_pool(name="consts", bufs=1) as consts,
            tc.tile_pool(name="encpp", bufs=2) as encpp,
            tc.tile_pool(name="encbp", bufs=1) as encbp,
            tc.tile_pool(name="hp", bufs=12) as hp,
            tc.tile_pool(name="outp", bufs=5) as outp,
            tc.tile_pool(name="psj", bufs=4, space="PSUM") as psj,
        ):
            # warm tile init on Pool — its queue is free earliest, and the
            # PE warmup matmuls can then start during the DMA-wait window
            warm = consts.tile([P, 4 * P], BF16, tag="warm")
            nc.gpsimd.memset(warm[:], 0.0)
            warm2 = consts.tile([P, P], BF16, tag="warm2")

            def load_slot(s, split=False, ring=None):
                t = encpp.tile([P, 4 * WAP], BF16, tag="encp")
                dma = (ring or nc.sync).dma_start
                if split:
                    dma(t[:, : 2 * WAP], encpA[s][:, : 2 * WAP])
                    dma(t[:, 2 * WAP :], encpA[s][:, 2 * WAP :])
                else:
                    dma(t[:], encpA[s])
                return t

            # critical first loads all on the SP HW ring (shortest preamble,
            # fastest); bulk B-section tiles go on the Pool SW ring
            enc_cur = None
            if SA:
                enc_cur = encpp.tile([P, 4 * WAP], BF16, tag="encp", name="enc_cur")
                nc.sync.dma_start(enc_cur[:, : 2 * WAP], encpA[0][:, : 2 * WAP])
            bvA_all = consts.tile([P, 4 * NA], F32, tag="bvA")
            nc.sync.dma_start(bvA_all[:], bvA[:, :])
            if SA:
                nc.sync.dma_start(enc_cur[:, 2 * WAP :], encpA[0][:, 2 * WAP :])
            bvA_sb = [bvA_all[:, jc * NA : (jc + 1) * NA] for jc in range(4)]
            wj2_all = consts.tile([P, 4 * V], BF16, tag="wj2")
            nc.gpsimd.dma_start(wj2_all[:], wj2[:, :])
            wj2_sb = [wj2_all[:, jc * V : (jc + 1) * V] for jc in range(4)]
            if SB:
                encb = encbp.tile([P, SB * 2048], BF16, tag="encpB")
                nbtb = encbp.tile([P, SB * 2048], BF16, tag="nbtB")
                for s in range(SB):
                    nc.gpsimd.dma_start(encb[:, s * 2048 : (s + 1) * 2048], encpB[s])
                    nc.gpsimd.dma_start(nbtb[:, s * 2048 : (s + 1) * 2048], nbtB[s])

            # ---- engine warmups during the DMA-wait window ----
            # ACT: a dummy Relu triggers the ~1.3us activation-table load
            # PE: dummy matmuls ramp the HAM clock gate (cold -> full speed)
            nc.scalar.memzero(warm2[:])
            nc.scalar.activation(
                warm2[:], warm2[:], mybir.ActivationFunctionType.Relu,
                bias=0.0, scale=1.0,
            )
            for i in range(NWARM):
                wps = psj.tile([P, 2 * 512], F32, tag="psj")
                nc.tensor.matmul(
                    wps[:, : 4 * P], warm[:, :P], warm[:], start=True, stop=True
                )

            # ---- static engine balancer (measured per-op costs) ----
            loads = {"v": 0.0, "s": 0.0, "g": 0.0}
            npool = [0]
            ecost = lambda tab: {"v": tab[0], "s": tab[1], "g": tab[2]}
            CO = {
                "ha": ecost(C_HA), "hb": ecost(C_HB),
                "epa": ecost(C_EPA), "epb": ecost(C_EPB),
            }

            def pick(kind, allowed=("v", "s", "g")):
                if npool[0] >= POOL_MAX:
                    allowed = tuple(e for e in allowed if e != "g")
                e = min(allowed, key=lambda x: loads[x] + CO[kind][x])
                loads[e] += CO[kind][e]
                if e == "g":
                    npool[0] += 1
                return e

            def h_op(dst, src, bias_ap, eng):
                if eng == "v":
                    nc.vector.tensor_scalar(
                        dst, src, bias_ap, 0.0,
                        mybir.AluOpType.add, mybir.AluOpType.max,
                    )
                elif eng == "g":
                    nc.gpsimd.tensor_scalar(
                        dst, src, bias_ap, 0.0,
                        mybir.AluOpType.add, mybir.AluOpType.max,
                    )
                else:
                    nc.scalar.activation(
                        dst, src, mybir.ActivationFunctionType.Relu,
                        bias=bias_ap, scale=1.0,
                    )

            def copy_on(eng, dst, src):
                if eng == "v":
                    nc.vector.tensor_copy(dst, src)
                else:
                    nc.scalar.copy(dst, src)

            def emit_b_group(s, g):
                # one fused max over the whole 4-item group: layout
                # [jc][ci][t] with enc replicated per ci on the host
                ht = hp.tile([P, 2048], BF16, tag="hb")
                loads["v"] += 1320.0
                nc.vector.tensor_tensor(
                    ht[:],
                    encb[:, s * 2048 : (s + 1) * 2048],
                    nbtb[:, s * 2048 : (s + 1) * 2048],
                    mybir.AluOpType.max,
                )
                ps = psj.tile([P, 2 * 512], F32, tag="psj")
                for jc in range(4):
                    nc.tensor.matmul(
                        ps[:, : 4 * P], wj2_sb[jc],
                        ht[:, jc * 512 : (jc + 1) * 512],
                        start=(jc == 0), stop=(jc == 3),
                    )
                gi = s * (CB // 4) + g
                ot = outp.tile([P, 4 * P], BF16, tag="outb")
                copy_on(pick("epb", ("v", "s")), ot[:], ps[:, : 4 * P])
                nc.sync.dma_start(outB[gi], ot[:])

            # B groups interleaved among A items (spread S-engine load)
            bqueue = [(s, g) for s in range(SB) for g in range(CB // 4)] if SB else []
            n_total_items = max(NITA, 1)
            bstep = max(1, n_total_items // (len(bqueue) + 1)) if bqueue else 0

            # ---- section A: width-3 slots, paired 2-bank PSUM epilogue ----
            # epilogues are emitted LAGGED so the in-order scalar engine never
            # head-of-line blocks a later item's Relu on an unfinished chain
            ps_pair = None
            ot_pair = None
            pending = []

            NPAIR_A = NITA // 2 if PAIRED else 0

            def flush_epi():
                ppair, opair, pidx = pending.pop(0)
                if pidx >= NPAIR_A - 2:
                    # tail pairs: split halves across V and S, and DMA each
                    # half as soon as its copy lands (latency, not throughput)
                    nc.vector.tensor_copy(opair[:, :WAP], ppair[:, :WAP])
                    nc.sync.dma_start(outA[pidx][:, :WAP], opair[:, :WAP])
                    nc.scalar.copy(opair[:, WAP:], ppair[:, 512 : 512 + WAP])
                    nc.sync.dma_start(outA[pidx][:, WAP:], opair[:, WAP:])
                else:
                    src_v = ppair[:].rearrange("p (g x) -> p g x", g=2)[:, :, :WAP]
                    dst_v = opair[:].rearrange("p (g x) -> p g x", g=2)
                    copy_on(pick("epa", ("v", "s")), dst_v, src_v)
                    nc.sync.dma_start(outA[pidx], opair[:])

            for s in range(SA):
                enc_next = None
                for c in range(CA):
                    if c == min(2, CA - 1) and s + 1 < SA:
                        enc_next = load_slot(s + 1)
                    idx = s * CA + c
                    h4 = []
                    for jc in range(4):
                        ht = hp.tile([P, WAP], BF16, tag=f"h{jc}")
                        eng = "v" if idx >= NITA - 1 else pick("ha")
                        h_op(
                            ht[:],
                            enc_cur[:, jc * WAP : (jc + 1) * WAP],
                            bvA_sb[jc][:, idx : idx + 1],
                            eng,
                        )
                        h4.append(ht)
                    if PAIRED:
                        half = idx % 2
                        if half == 0:
                            ps_pair = psj.tile([P, 2 * 512], F32, tag="psj")
                            ot_pair = outp.tile([P, 2 * WAP], BF16, tag="out")
                        dst = ps_pair[:, half * 512 : half * 512 + WAP]
                        for jc in range(4):
                            nc.tensor.matmul(
                                dst, wj2_sb[jc], h4[jc][:],
                                start=(jc == 0), stop=(jc == 3),
                            )
                        if PAIR_DUMMY:
                            nc.tensor.matmul(
                                ps_pair[:, 896:1024], warm[:, :P], warm[:, :P],
                                start=True, stop=True,
                            )
                        if half == 1:
                            pending.append((ps_pair, ot_pair, idx // 2))
                            if len(pending) > 1:
                                flush_epi()
                    else:
                        ps = psj.tile([P, 2 * 512], F32, tag="psj")
                        for jc in range(4):
                            nc.tensor.matmul(
                                ps[:, :WAP], wj2_sb[jc], h4[jc][:],
                                start=(jc == 0), stop=(jc == 3),
                            )
                        ot = outp.tile([P, WAP], BF16, tag="out")
                        copy_on(pick("epb", ("v", "s")), ot[:], ps[:, :WAP])
                        nc.sync.dma_start(outA[idx], ot[:])
                    if bqueue and bstep and idx % bstep == bstep - 1:
                        emit_b_group(*bqueue.pop(0))
                if enc_next is not None:
                    enc_cur = enc_next

            while pending:
                flush_epi()
            while bqueue:
                emit_b_group(*bqueue.pop(0))
            if os.environ.get("KERNEL_DEBUG"):
                print(f"balancer loads: {loads} pool_ops={npool[0]}")
    _split_excess_waits(nc)
    return nc


def _host_bvec(targets, emb, W1, b1, W2, b2, Wj1, bj1):
    """Prediction network on host -> bvec[b, u, JOIN] (pred_proj + bj1)."""
    tgt = np.asarray(targets).astype(np.int64)
    ext = np.pad(tgt, ((0, 0), (H, 0)), constant_values=V - 1)  # [B, U+H]
    ctx0 = ext[:, 1 : 1 + NU]
    ctx1 = ext[:, 0:NU]
    e = np.concatenate([emb[ctx0], emb[ctx1]], axis=-1)  # [B, NU, H*EMB]
    p = np.maximum(e @ W1 + b1, 0.0)
    pred = np.maximum(p @ W2 + b2, 0.0)  # [B, NU, PRED]
    Wp = Wj1[ENC:]
    return (pred @ Wp + bj1).astype(np.float32)  # [B, NU, JOIN]


def _schedule(enc_sizes, tgt_sizes):
    """Decompose the ragged grid into width-3 / width-1 chunk work and
    LPT-pack it onto 8 cores.  Returns (SA, CA, SB, CB, cores, leftover):
    cores[i] = {"aslots": [(b,t0,w)], "agrid": [[item or None]*CA]*SA,
                "bslots": [(b,t0,w)], "bgrid": ...}; item = (b, t0, w, u);
    leftover = [(b, t0, w, u)] to compute on the host."""
    w3, w1 = [], []  # chunks: (b, t0, width, ucnt)
    for b in range(B):
        ttiles = max(1, math.ceil(int(enc_sizes[b]) / P))
        ucnt = int(tgt_sizes[b]) + 1
        t = 0
        while ttiles - t >= 3:
            w3.append((b, t * P, 3, ucnt))
            t += 3
        rem = ttiles - t
        if rem == 2:
            w3.append((b, t * P, 2, ucnt))  # padded into a width-3 slot
        elif rem == 1:
            w1.append((b, t * P, 1, ucnt))

    n3 = sum(c[3] for c in w3)
    n1 = sum(c[3] for c in w1)
    CA = 11
    CB = 4

    total_units = 3.0 * n3 + 1.0 * n1
    target = total_units / 8.0

    def pack(chunks, S, C, loads, weight):
        cores = [
            {"slots": [], "grid": [[None] * C for _ in range(S)], "items": 0}
            for _ in range(8)
        ]
        leftover = []
        for b, t0, w, n in sorted(chunks, key=lambda c: -c[3]):
            u0 = 0
            left = n
            while left > 0:
                order = sorted(range(8), key=lambda i: loads[i])
                placed = False
                for i in order:
                    cc = cores[i]
                    cap = (S - len(cc["slots"])) * C
                    if cap <= 0:
                        continue
                    # don't let one core grab far more than its fair share
                    fair = max(C, int(round((target - loads[i]) / weight / C)) * C)
                    take = min(left, cap, fair)
                    nslots = math.ceil(take / C)
                    base = len(cc["slots"])
                    for j in range(take):
                        si = base + j // C
                        cc["grid"][si][j % C] = (b, t0, w, u0 + j)
                    for _ in range(nslots):
                        cc["slots"].append((b, t0, w))
                    cc["items"] += take
                    loads[i] += take * weight
                    u0 += take
                    left -= take
                    placed = True
                    break
                if not placed:
                    for j in range(left):
                        leftover.append((b, t0, w, u0 + j))
                    break
        return cores, leftover

    # device time is proportional to grid CAPACITY (every cell is computed),
    # so try configs in increasing total-cost order and accept the first
    # whose unpacked remainder is small enough to compute on the host.
    SA0 = max(1, math.ceil((n3 / 8) / CA))
    SB0 = min(3, math.ceil((n1 / 8) / CB)) if n1 else 0
    configs = []
    for da in range(3):
        for db in range(3):
            SA_t = SA0 + da
            SB_t = min(3, SB0 + db) if n1 else 0
            cost = SA_t * CA * 3 + SB_t * CB
            configs.append((cost, SA_t, SB_t))
    configs = sorted(set(configs))
    if os.environ.get("KERNEL_FORCE_SA"):
        fsa = int(os.environ["KERNEL_FORCE_SA"])
        fsb = int(os.environ.get("KERNEL_FORCE_SB", SB0 or 0))
        configs = [(0, fsa, fsb)]
    best = None
    for cost, SA, SB in configs:
        loads = [0.0] * 8
        acores, aleft = pack(w3, SA, CA, loads, 3.0)
        if SB:
            bcores, bleft = pack(w1, SB, CB, loads, 1.0)
        else:
            bcores = [{"slots": [], "grid": [], "items": 0} for _ in range(8)]
            bleft = []
        nleft = len(aleft) + len(bleft)
        cand = (nleft, SA, SB, acores, bcores, aleft + bleft)
        if best is None or cand[0] < best[0]:
            best = cand
        if nleft <= 18:  # small host fallback is cheaper than a bigger grid
            break
    _, SA, SB, acores, bcores, leftover = best
    cores = []
    for i in range(8):
        cores.append({
            "aslots": acores[i]["slots"], "agrid": acores[i]["grid"],
            "bslots": bcores[i]["slots"], "bgrid": bcores[i]["grid"],
        })
    return SA, CA, SB, CB, cores, leftover


def _get_compiled(key):
    if key not in _CACHE:
        _CACHE[key] = _build_nc(*key)
    return _CACHE[key]


def kernel(
    encoder_states,
    encoder_states_size,
    targets,
    targets_size,
    emb,
    W1,
    b1,
    W2,
    b2,
    Wj1,
    bj1,
    Wj2,
    bj2,
):
    import ml_dtypes

    enc = np.ascontiguousarray(np.asarray(encoder_states, dtype=np.float32))
    enc_sizes = np.asarray(encoder_states_size).astype(np.int64)
    tgt_sizes = np.asarray(targets_size).astype(np.int64)
    emb = np.asarray(emb, dtype=np.float32)
    W1 = np.asarray(W1, dtype=np.float32)
    b1 = np.asarray(b1, dtype=np.float32)
    W2 = np.asarray(W2, dtype=np.float32)
    b2 = np.asarray(b2, dtype=np.float32)
    Wj1 = np.asarray(Wj1, dtype=np.float32)
    bj1 = np.asarray(bj1, dtype=np.float32)
    Wj2 = np.ascontiguousarray(np.asarray(Wj2, dtype=np.float32))
    bj2 = np.asarray(bj2, dtype=np.float32)

    bf16 = ml_dtypes.bfloat16
    bvec = _host_bvec(targets, emb, W1, b1, W2, b2, Wj1, bj1)
    We = np.ascontiguousarray(Wj1[:ENC])
    SA, CA, SB, CB, cores, leftover = _schedule(enc_sizes, tgt_sizes)
    WA = 3
    WAP = WA * P

    nc = _get_compiled((SA, CA, SB, CB))

    trace = bool(os.environ.get("KERNEL_TRACE"))
    if trace:
        _install_ntff_hook()

    # host encoder projection: encp[b] = enc[b] @ We -> transposed [4, 128, T]
    encp = np.matmul(enc, We)  # [B, T, JOIN] fp32
    encpT = np.ascontiguousarray(encp.transpose(0, 2, 1)).reshape(B, 4, P, T)
    encpT_c = encpT.astype(bf16)
    Wj2_c = Wj2.astype(bf16)
    bvec_c = bvec  # [B, NU, JOIN] fp32 (tensor_scalar needs fp32 scalar)
    # B section max-route correction: corr[b,u,:] = bf16(bvec) @ bf16(Wj2)
    bvec_b16 = bvec.astype(bf16)
    corr = np.matmul(bvec_b16.astype(np.float32), Wj2_c.astype(np.float32))

    NITA = SA * CA
    NA = max(NITA, 1)
    WAP = WA * P
    # pre-arranged flat layouts: every DMA is [128, free] contiguous
    wj2_flat = np.ascontiguousarray(
        Wj2_c.reshape(4, P, V).transpose(1, 0, 2).reshape(P, 4 * V)
    )
    in_maps = []
    for core in cores:
        encpA_arr = np.zeros((SA, P, 4 * WAP), dtype=bf16)
        for si, (b, t0, w) in enumerate(core["aslots"]):
            wid = w * P
            for jc in range(4):
                encpA_arr[si, :, jc * WAP : jc * WAP + wid] = encpT_c[
                    b, jc, :, t0 : t0 + wid
                ]
        bvA_arr = np.zeros((P, 4 * NA), dtype=np.float32)
        for si in range(SA):
            for c in range(CA):
                it = core["agrid"][si][c]
                if it is None:
                    continue
                b, t0, w, u = it
                bv4 = bvec[b, u].reshape(4, P)
                for jc in range(4):
                    bvA_arr[:, jc * NA + si * CA + c] = bv4[jc]
        m = {
            "encpA": encpA_arr,
            "bvA": bvA_arr,
            "wj2": wj2_flat,
        }
        if SB:
            # layout per slot: [jc][ci][t]; enc replicated across ci, bias
            # (-bf16(bvec)) broadcast across t
            encpB_arr = np.zeros((SB, P, 2048), dtype=bf16)
            nbtB_arr = np.zeros((SB, P, 2048), dtype=bf16)
            for si, (b, t0, w) in enumerate(core["bslots"]):
                for jc in range(4):
                    blk = encpT_c[b, jc, :, t0 : t0 + P]  # [P, 128]
                    for ci in range(4):
                        encpB_arr[si, :, jc * 512 + ci * P : jc * 512 + (ci + 1) * P] = blk
            for si in range(SB):
                for c in range(CB):
                    it = core["bgrid"][si][c]
                    if it is None:
                        continue
                    b, t0, w, u = it
                    nb4 = -bvec_b16[b, u].reshape(4, P)  # [jc, p]
                    for jc in range(4):
                        nbtB_arr[si, :, jc * 512 + c * P : jc * 512 + (c + 1) * P] = (
                            nb4[jc][:, None]
                        )
            m["encpB"] = encpB_arr
            m["nbtB"] = nbtB_arr
        in_maps.append(m)

    kwargs = {}
    if trace:
        kwargs = dict(trace=True, trace_cores=list(range(8)))
    res = None
    last_exc = None
    for attempt in range(3):
        try:
            res = bass_utils.run_bass_kernel_spmd(
                nc, in_maps, core_ids=list(range(8)), **kwargs
            )
            break
        except Exception as e:  # transient device wedges happen; retry
            last_exc = e
            import time as _time

            _time.sleep(2.0)
    if res is None:
        raise last_exc
    kernel.last_results = [res]

    final = np.zeros((B, T, NU, V), dtype=np.float32)
    for ki, core in enumerate(cores):
        outA = np.asarray(res.results[ki]["outA"]).astype(np.float32)
        if NITA % 2 == 0 and NITA > 0:
            outA = outA.reshape(NITA // 2, P, 2, WA * P).transpose(0, 2, 1, 3).reshape(
                NITA, P, WA * P
            )
        for si in range(SA):
            for c in range(CA):
                it = core["agrid"][si][c]
                if it is None:
                    continue
                b, t0, w, u = it
                rows = min(w * P, int(enc_sizes[b]) - t0)
                if rows <= 0:
                    continue
                final[b, t0 : t0 + rows, u, :] = outA[si * CA + c, :, :rows].T + bj2
        if SB:
            outB = np.asarray(res.results[ki]["outB"]).astype(np.float32)
            for si in range(SB):
                for c in range(CB):
                    it = core["bgrid"][si][c]
                    if it is None:
                        continue
                    b, t0, w, u = it
                    rows = min(P, int(enc_sizes[b]) - t0)
                    if rows <= 0:
                        continue
                    gi = si * (CB // 4) + c // 4
                    ci = c % 4
                    final[b, t0 : t0 + rows, u, :] = (
                        outB[gi, :, ci * P : ci * P + rows].T + (bj2 + corr[b, u])
                    )

    # host fallback for anything that didn't fit the device grids
    if leftover:
        bychunk = {}
        for b, t0, w, u in leftover:
            bychunk.setdefault((b, t0, w), []).append(u)
        for (b, t0, w), us in bychunk.items():
            rows = min(w * P, int(enc_sizes[b]) - t0)
            if rows <= 0:
                continue
            ep = encp[b, t0 : t0 + rows, :]  # [rows, JOIN] fp32
            for u in us:
                hh = np.maximum(ep + bvec[b, u], 0.0)
                final[b, t0 : t0 + rows, u, :] = hh @ Wj2 + bj2

    return final

